# revision 1
# baseline (speedup 1.0000x reference)
"""ArteryMixer Trainium2 kernel v12: v1 pipeline structure + fp8 DoubleRow GEMMs.

Per-token math (B=2,S=2048,A=8,R=8,DIM=1024,H=8,HD=128,SC=16):
  qkv = concat(x+emb, res) @ Wqkv.T ; q,k rmsnorm ; k_res roped (folded into W);
  scores=elu(q@k.T/sqrt(HD)) ; mixed = scores@v/16 ; out = x + scale*(mixed@Wproj.T)

Deltas vs the HW-proven v1 schedule (which this keeps exactly):
  - artery_embed folded into the fp8 GEMM operand on host (xe8 = fp8(x+emb));
    bias adds on device are gone. Residual add keeps a separate bf16 x.
  - QKV / V / proj GEMMs in fp8e4m3 with perf_mode=DoubleRow (half the PE
    cycles); weights scaled x16 on host to dodge fp8 subnormals; descale is
    folded into the rmsnorm eps (exact) and mscale (v/proj path, /256).
  - mixedT stored fp8 (feeds the fp8 proj GEMM).
  - attention matmuls stay bf16 (FD=128 is below DoubleRow's win threshold).
"""

import numpy as np
import ml_dtypes

bf16 = ml_dtypes.bfloat16
f8e4 = ml_dtypes.float8_e4m3fn

HEADS = 8
HD = 128
DIM = 1024
MD = 1024
A = 8
RKV = 8
SC = 16
EPS = 1.1920929e-07
ROPE_BASE = 10000.0
N_CORES = 8
B, S = 2, 2048
TOK_PER_CORE = (B * S) // N_CORES  # 512
BLK_TOK = 64                        # tokens per pipeline block
NB = TOK_PER_CORE // BLK_TOK        # 8 blocks
CPB = BLK_TOK * 8                   # 512 cols per block (token-major, slot-minor)
WSCALE = 16.0                       # host-side fp8 weight scale


def _rope_matrix():
    inv_freq = 1.0 / (ROPE_BASE ** (np.arange(0, HD, 2, dtype=np.float64) / HD))
    c, s = np.cos(inv_freq), np.sin(inv_freq)
    Rm = np.zeros((HD, HD), dtype=np.float64)
    i = np.arange(HD // 2)
    # reference _rope: out1 = x1*c + x2*s ; out2 = -x1*s + x2*c
    Rm[i, i] = c
    Rm[i, i + 64] = s
    Rm[i + 64, i] = -s
    Rm[i + 64, i + 64] = c
    return Rm


def build_program(tok_per_core=TOK_PER_CORE, repeat=1):
    import concourse.bass as bass  # noqa
    import concourse.mybir as mybir
    import concourse.tile as tile
    from concourse import bacc
    from concourse import bass_isa

    dt = mybir.dt
    Alu = mybir.AluOpType
    Act = mybir.ActivationFunctionType
    DR = mybir.MatmulPerfMode.DoubleRow

    nb = tok_per_core // BLK_TOK
    COLS = tok_per_core * 8

    nc = bacc.Bacc(None, target_bir_lowering=False)

    xe8_t = nc.dram_tensor("xe8_t", [DIM, COLS], dt.float8e4, kind="ExternalInput")
    xr8_t = nc.dram_tensor("xr8_t", [DIM, COLS], dt.float8e4, kind="ExternalInput")
    xab_t = nc.dram_tensor("xab_t", [DIM, COLS], dt.bfloat16, kind="ExternalInput")
    wqkv_t = nc.dram_tensor("wqkv_t", [DIM, 3 * MD], dt.float8e4, kind="ExternalInput")
    wv_t = nc.dram_tensor("wv_t", [DIM, MD], dt.float8e4, kind="ExternalInput")
    wproj_t = nc.dram_tensor("wproj_t", [MD, DIM], dt.float8e4, kind="ExternalInput")
    mask_d = nc.dram_tensor("mask", [128, 128], dt.bfloat16, kind="ExternalInput")
    mscale_d = nc.dram_tensor("mscale", [128, 8], dt.float32, kind="ExternalInput")
    out_t = nc.dram_tensor("out_t", [DIM, COLS], dt.bfloat16, kind="ExternalOutput")

    with tile.TileContext(nc) as tc:
        with (
            tc.tile_pool(name="w", bufs=1) as wpool,
            tc.tile_pool(name="x", bufs=2) as xpool,
            tc.tile_pool(name="slab", bufs=2) as spool,
            tc.tile_pool(name="vslab", bufs=1) as vpool,
            tc.tile_pool(name="nrm", bufs=2) as npool,
            tc.tile_pool(name="att", bufs=2) as fpool,
            tc.tile_pool(name="rtp", bufs=3) as rtpool,
            tc.tile_pool(name="y", bufs=2) as ypool,
            tc.tile_pool(name="mm", bufs=2, space="PSUM") as mmpool,
            tc.tile_pool(name="sc", bufs=2, space="PSUM") as scpool,
        ):
            # Preload the combined ln+exp activation table set (set 6,
            # 'natural_log_exp_and_others'): every ACT function used below
            # (Ln, Exp, Relu, Copy) lives in it, so no table reloads ever.
            nc.scalar.add_instruction(mybir.InstLoadActFuncSet(
                name=nc.get_next_instruction_name(), act_func_set_id=6,
                ins=[], outs=[]))

            # ---- resident weights/constants ----
            wqkv_sb = wpool.tile([128, 8, 3 * MD], dt.float8e4)
            nc.sync.dma_start(
                wqkv_sb, wqkv_t[:].rearrange("(dc p) f -> p dc f", p=128)
            )
            wv_sb = wpool.tile([128, 8, MD], dt.float8e4)
            nc.sync.dma_start(wv_sb, wv_t[:].rearrange("(dc p) f -> p dc f", p=128))
            wproj_sb = wpool.tile([128, 8, DIM], dt.float8e4)
            nc.sync.dma_start(
                wproj_sb, wproj_t[:].rearrange("(mc p) f -> p mc f", p=128)
            )
            mask_sb = wpool.tile([128, 128], dt.bfloat16)
            nc.sync.dma_start(mask_sb, mask_d[:])
            mscale_sb = wpool.tile([128, 8], dt.float32)
            nc.sync.dma_start(mscale_sb, mscale_d[:])

            xe_dram = xe8_t[:].rearrange("(dc p) c -> p dc c", p=128)
            xr_dram = xr8_t[:].rearrange("(dc p) c -> p dc c", p=128)
            xa_dram = xab_t[:].rearrange("(dc p) c -> p dc c", p=128)
            yo_dram = out_t[:].rearrange("(dc p) c -> p dc c", p=128)

            def build_gemm_items(blk):
                """Allocate block tiles + return GEMM work-item closures."""
                c0 = blk * CPB
                xe8 = xpool.tile([128, 8, CPB], dt.float8e4, tag="xe8", name="xe8")
                xr8 = xpool.tile([128, 8, CPB], dt.float8e4, tag="xr8", name="xr8")
                xab = xpool.tile([128, 8, CPB], dt.bfloat16, tag="xab", name="xab")
                nc.sync.dma_start(xe8, xe_dram[:, :, c0 : c0 + CPB])
                nc.sync.dma_start(xr8, xr_dram[:, :, c0 : c0 + CPB])
                nc.sync.dma_start(xab, xa_dram[:, :, c0 : c0 + CPB])
                qT = spool.tile([128, 8, CPB], dt.bfloat16, tag="qT")
                kTa = spool.tile([128, 8, CPB], dt.bfloat16, tag="kTa")
                kTr = spool.tile([128, 8, CPB], dt.bfloat16, tag="kTr")
                va = vpool.tile([128, 4, 8, HD], dt.bfloat16, tag="va")
                vr = vpool.tile([128, 4, 8, HD], dt.bfloat16, tag="vr")
                st = dict(xe8=xe8, xr8=xr8, xab=xab, qT=qT, kTa=kTa, kTr=kTr,
                          va=va, vr=vr, c0=c0)
                slabs = [qT, kTa, kTr]
                items = []

                def qkv_item(fp):
                    # computes fc = 2*fp, 2*fp+1 (same slab)
                    def go():
                        ps = mmpool.tile([128, 2, 512], dt.float32, tag="mmps")
                        for half in range(2):
                            fc = 2 * fp + half
                            src = xr8 if fc >= 16 else xe8
                            for j in range(4):
                                nc.tensor.matmul(
                                    ps[:, half, :],
                                    wqkv_sb[:, 2 * j : 2 * j + 2,
                                            fc * 128 : (fc + 1) * 128],
                                    src[:, 2 * j : 2 * j + 2, :],
                                    start=(j == 0),
                                    stop=(j == 3),
                                    perf_mode=DR,
                                )
                        fc0 = 2 * fp
                        dst = slabs[fc0 // 8][:, fc0 % 8 : fc0 % 8 + 2, :]
                        nc.vector.tensor_copy(dst, ps)
                    return go

                def v_item(isart, rc):
                    def go():
                        src, dstv = (xe8, va) if isart else (xr8, vr)
                        ps = mmpool.tile([128, 2, 512], dt.float32, tag="mmps")
                        for vh in range(2):
                            for j in range(4):
                                nc.tensor.matmul(
                                    ps[:, vh, :],
                                    src[:, 2 * j : 2 * j + 2,
                                        rc * 128 : (rc + 1) * 128],
                                    wv_sb[:, 2 * j : 2 * j + 2,
                                          vh * 512 : (vh + 1) * 512],
                                    start=(j == 0),
                                    stop=(j == 3),
                                    perf_mode=DR,
                                )
                        dv = dstv[:, rc, :, :]
                        nc.scalar.copy(out=dv, in_=ps)
                    return go

                for fp in range(12):
                    items.append(qkv_item(fp))
                for isart in (True, False):
                    for rc in range(4):
                        items.append(v_item(isart, rc))
                return st, items

            def build_attn_items(st):
                """Work items for norm + attention + proj of a block."""
                qT, kTa, kTr = st["qT"], st["kTa"], st["kTr"]
                va, vr, xab, c0 = st["va"], st["vr"], st["xab"], st["c0"]
                items = []

                def norm_item(slab, scv, hp):
                    # normalizes head-chunks 2*hp, 2*hp+1 of slab; rsqrt as
                    # exp(-0.5*ln(m)) so ACT never leaves table set 6 (eps is
                    # dropped: it is ~6e-8 relative to ssq and unobservable).
                    def go():
                        with nc.allow_low_precision(
                            reason="all-reduce upcasts internally; bf16 ~0.4%"
                        ):
                            sl = slab[:, 2 * hp : 2 * hp + 2, :]
                            sq = npool.tile([128, 2, CPB], dt.bfloat16, tag="nsq")
                            nc.vector.tensor_mul(sq, sl, sl)
                            ssq = npool.tile([128, 2, CPB], dt.bfloat16, tag="nssq")
                            # two half-width all-reduces beat one wide one:
                            # the DSP all-reduce cost grows superlinearly in
                            # free size.
                            for hf in range(2):
                                nc.gpsimd.partition_all_reduce(
                                    ssq[:, hf, :], sq[:, hf, :], channels=128,
                                    reduce_op=bass_isa.ReduceOp.add,
                                )
                            lnm = npool.tile([128, 2, CPB], dt.bfloat16, tag="nsq")
                            nc.scalar.activation(lnm, ssq, Act.Ln, scale=scv)
                            rs = npool.tile([128, 2, CPB], dt.bfloat16, tag="nssq")
                            nc.scalar.activation(rs, lnm, Act.Exp, scale=-0.5)
                            nc.vector.tensor_mul(sl, sl, rs)
                    return go

                mixedT = vpool.tile([128, 8, CPB], dt.float8e4, tag="mixedT")
                st["mixedT"] = mixedT
                routes = {}

                def scores_item(g, half):
                    def go():
                        gsl = slice(g * 128, (g + 1) * 128)
                        kT = kTa if half == 0 else kTr
                        ps = scpool.tile([128, 8, 128], dt.float32, tag="scps")
                        for h in range(8):
                            nc.tensor.matmul(
                                ps[:, h, :], kT[:, h, gsl], qT[:, h, gsl],
                                start=True, stop=True,
                            )
                        esc = fpool.tile([128, 8, 128], dt.bfloat16, tag="ers")
                        rsc = fpool.tile([128, 8, 128], dt.bfloat16, tag="ers")
                        nc.scalar.activation(esc, ps, Act.Exp)
                        nc.scalar.activation(rsc, ps, Act.Relu)
                        # elu = relu(s) + (min(exp(s),1) - 1)
                        nc.vector.tensor_scalar(esc, esc, 1.0, -1.0, Alu.min, Alu.add)
                        nc.vector.tensor_add(esc, rsc, esc)
                        route = rtpool.tile([128, 8, 128], dt.bfloat16, tag="rt")
                        nc.vector.tensor_mul(
                            route, esc,
                            mask_sb[:, None, :].to_broadcast((128, 8, 128)),
                        )
                        routes[(g, half)] = route
                    return go

                def mixed_item(g):
                    def go():
                        gsl = slice(g * 128, (g + 1) * 128)
                        mx = scpool.tile([128, 8, 128], dt.float32, tag="scps")
                        for h in range(8):
                            nc.tensor.matmul(
                                mx[:, h, :], va[:, g, h, :],
                                routes[(g, 0)][:, h, :], start=True, stop=False,
                            )
                            nc.tensor.matmul(
                                mx[:, h, :], vr[:, g, h, :],
                                routes[(g, 1)][:, h, :], start=False, stop=True,
                            )
                        nc.scalar.copy(out=mixedT[:, :, gsl], in_=mx)
                    return go

                def proj_item(dp):
                    # projects feature chunks dc = 2*dp, 2*dp+1
                    def go():
                        ps = mmpool.tile([128, 2, 512], dt.float32, tag="mmps")
                        yb = ypool.tile([128, 2, CPB], dt.bfloat16, tag="yb")
                        for half in range(2):
                            dc = 2 * dp + half
                            for j in range(4):
                                nc.tensor.matmul(
                                    ps[:, half, :],
                                    wproj_sb[:, 2 * j : 2 * j + 2,
                                             dc * 128 : (dc + 1) * 128],
                                    mixedT[:, 2 * j : 2 * j + 2, :],
                                    start=(j == 0), stop=(j == 3),
                                    perf_mode=DR,
                                )
                            nc.vector.scalar_tensor_tensor(
                                out=yb[:, half, :], in0=ps[:, half, :],
                                scalar=mscale_sb[:, dc : dc + 1],
                                in1=xab[:, dc, :],
                                op0=Alu.mult, op1=Alu.add,
                            )
                        nc.sync.dma_start(
                            yo_dram[:, 2 * dp : 2 * dp + 2, c0 : c0 + CPB], yb
                        )
                    return go

                norm_items = []
                for slab, scv in (
                    (qT, 1.0 / HD),
                    (kTa, 1.0),
                    (kTr, 1.0),
                ):
                    for hp in range(4):
                        norm_items.append(norm_item(slab, scv, hp))
                return dict(
                    norm=norm_items,
                    groups=[(scores_item(g, 0), scores_item(g, 1), mixed_item(g))
                            for g in range(4)],
                    proj=[proj_item(dp) for dp in range(4)],
                )

            def merge(attn, gemm):
                """Structured interleave: norm 3:1 with gemms, then per group
                sc,G,sc,G,mx,G, then proj 1:1 with gemms; leftovers last."""
                out = []
                gq = list(gemm)

                def g(n):
                    for _ in range(n):
                        if gq:
                            out.append(gq.pop(0))

                if attn is None:
                    return list(gemm)
                for i, it in enumerate(attn["norm"]):
                    out.append(it)
                    if i % 3 == 2:
                        g(1)
                for sc0, sc1, mx in attn["groups"]:
                    out.append(sc0); g(1)
                    out.append(sc1); g(1)
                    out.append(mx); g(1)
                for p in attn["proj"]:
                    out.append(p); g(1)
                out.extend(gq)
                return out

            blklist = [b for _ in range(repeat) for b in range(nb)]
            prev_st = None
            for i in range(len(blklist) + 1):
                gemm_items = []
                if i < len(blklist):
                    st, gemm_items = build_gemm_items(blklist[i])
                attn = build_attn_items(prev_st) if prev_st is not None else None
                for item in merge(attn, gemm_items):
                    item()
                if i < len(blklist):
                    prev_st = st

    nc.compile()
    return nc


def host_prep(x, artery_embed, residual_kv, Wqkv, Wproj, mixer_scale,
              tok_per_core=TOK_PER_CORE, n_cores=N_CORES):
    T = x.shape[0] * x.shape[1]
    x_flat = np.asarray(x, dtype=np.float32).reshape(T, A, DIM)
    res_flat = np.asarray(residual_kv, dtype=np.float32).reshape(T, RKV, DIM)
    emb = np.asarray(artery_embed, dtype=np.float32)
    xe_flat = x_flat + emb[None]

    Rm = _rope_matrix()
    Wq = np.asarray(Wqkv[0:MD], dtype=np.float64)
    Wk = np.asarray(Wqkv[MD : 2 * MD], dtype=np.float64)
    Wv = np.asarray(Wqkv[2 * MD : 3 * MD], dtype=np.float64)
    Wk_res = np.einsum("de,hec->hdc", Rm, Wk.reshape(HEADS, HD, DIM)).reshape(MD, DIM)

    wqkv_t = np.ascontiguousarray(
        np.concatenate([Wq, Wk, Wk_res], axis=0).T * WSCALE
    ).astype(f8e4)
    wv_t = np.ascontiguousarray(Wv.T * WSCALE).astype(f8e4)
    wproj_t = np.ascontiguousarray(
        np.asarray(Wproj, dtype=np.float64).T * WSCALE
    ).astype(f8e4)

    mask = np.zeros((128, 128), dtype=np.float32)
    for t in range(16):
        mask[t * 8 : (t + 1) * 8, t * 8 : (t + 1) * 8] = 1.0 / SC
    mask = mask.astype(bf16)

    # v path and proj each carry WSCALE; descale both via mscale.
    mscale = np.ascontiguousarray(
        (np.asarray(mixer_scale, dtype=np.float32) / (WSCALE * WSCALE))
        .reshape(8, 128).T
    )

    shared = dict(
        wqkv_t=wqkv_t, wv_t=wv_t, wproj_t=wproj_t, mask=mask, mscale=mscale,
    )
    in_maps = []
    for i in range(n_cores):
        sl = slice(i * tok_per_core, (i + 1) * tok_per_core)
        xe = np.ascontiguousarray(
            xe_flat[sl].reshape(tok_per_core * A, DIM).T
        ).astype(f8e4)
        xr = np.ascontiguousarray(
            res_flat[sl].reshape(tok_per_core * RKV, DIM).T
        ).astype(f8e4)
        xa = np.ascontiguousarray(
            x_flat[sl].reshape(tok_per_core * A, DIM).T
        ).astype(bf16)
        m = dict(shared)
        m["xe8_t"] = xe
        m["xr8_t"] = xr
        m["xab_t"] = xa
        in_maps.append(m)
    return in_maps


def assemble_output(outs, tok_per_core=TOK_PER_CORE):
    """outs: list of (DIM, tok_per_core*8) bf16 arrays -> (B,S,A,DIM) f32."""
    parts = []
    for o in outs:
        y = np.asarray(o, dtype=np.float32)  # (1024, T*8)
        parts.append(y.reshape(DIM, tok_per_core, A).transpose(1, 2, 0))
    full = np.concatenate(parts, axis=0)  # (n_tok, A, DIM)
    if full.shape[0] == B * S:
        full = full.reshape(B, S, A, DIM)
    return np.ascontiguousarray(full)


_NC_CACHE = {}


def kernel(x, artery_embed, residual_kv, Wqkv, Wproj, mixer_scale):
    from concourse.bass_utils import run_bass_kernel_spmd

    key = TOK_PER_CORE
    if key not in _NC_CACHE:
        _NC_CACHE[key] = build_program(TOK_PER_CORE)
    nc = _NC_CACHE[key]

    in_maps = host_prep(x, artery_embed, residual_kv, Wqkv, Wproj, mixer_scale)
    res = run_bass_kernel_spmd(nc, in_maps, core_ids=list(range(N_CORES)))
    outs = [r["out_t"] for r in res.results]
    return assemble_output(outs)



# revision 8
# speedup vs baseline: 1.1677x; 1.1677x over previous
"""ArteryMixer Trainium2 kernel v13: v12 fp8 DoubleRow GEMMs + matmul-based
rmsnorm (no GPSIMD all-reduce).

Per-token math (B=2,S=2048,A=8,R=8,DIM=1024,H=8,HD=128,SC=16):
  qkv = concat(x+emb, res) @ Wqkv.T ; q,k rmsnorm ; k_res roped (folded into W);
  scores=elu(q@k.T/sqrt(HD)) ; mixed = scores@v/16 ; out = x + scale*(mixed@Wproj.T)

Deltas vs v12:
  - rmsnorm sum-of-squares over the partition (HD) dim via TensorE ones-column
    matmuls into a [24,512] PSUM tile (one row per 128-feature chunk), instead
    of 24 GPSIMD partition_all_reduce calls per block (~2-3us each - v12's
    bottleneck engine).  rsqrt = exp(-0.5*ln(.)) batched over the whole block
    in two small ACT ops with a per-partition scale vector (1/HD for q rows).
    The per-column scale is broadcast back across partitions with a K=1
    ones-row matmul and applied by one DVE multiply per chunk.
  - scores/mixed PSUM tiles shrunk to [128,4,128] (1 bank) so PSUM fits:
    mm 2x2 banks + scores 3x1 + ssq 1 = 8 banks.
  - qkv evacuation moved to ACT (PSUM-source is cheaper there); the square
    for the norm is a DVE bf16 multiply on the evacuated slab.
"""

import numpy as np
import ml_dtypes

bf16 = ml_dtypes.bfloat16
f8e4 = ml_dtypes.float8_e4m3fn

HEADS = 8
HD = 128
DIM = 1024
MD = 1024
A = 8
RKV = 8
SC = 16
EPS = 1.1920929e-07
ROPE_BASE = 10000.0
N_CORES = 8
B, S = 2, 2048
TOK_PER_CORE = (B * S) // N_CORES  # 512
BLK_TOK = 64                        # tokens per pipeline block
NB = TOK_PER_CORE // BLK_TOK        # 8 blocks
CPB = BLK_TOK * 8                   # 512 cols per block (token-major, slot-minor)
WSCALE = 16.0                       # host-side fp8 weight scale


def _rope_matrix():
    inv_freq = 1.0 / (ROPE_BASE ** (np.arange(0, HD, 2, dtype=np.float64) / HD))
    c, s = np.cos(inv_freq), np.sin(inv_freq)
    Rm = np.zeros((HD, HD), dtype=np.float64)
    i = np.arange(HD // 2)
    # reference _rope: out1 = x1*c + x2*s ; out2 = -x1*s + x2*c
    Rm[i, i] = c
    Rm[i, i + 64] = s
    Rm[i + 64, i] = -s
    Rm[i + 64, i + 64] = c
    return Rm


def build_program(tok_per_core=TOK_PER_CORE, repeat=1):
    import concourse.bass as bass  # noqa
    import concourse.mybir as mybir
    import concourse.tile as tile
    from concourse import bacc

    dt = mybir.dt
    Alu = mybir.AluOpType
    Act = mybir.ActivationFunctionType
    DR = mybir.MatmulPerfMode.DoubleRow

    nb = tok_per_core // BLK_TOK
    COLS = tok_per_core * 8

    nc = bacc.Bacc(None, target_bir_lowering=False)

    xe8_t = nc.dram_tensor("xe8_t", [DIM, COLS], dt.float8e4, kind="ExternalInput")
    xr8_t = nc.dram_tensor("xr8_t", [DIM, COLS], dt.float8e4, kind="ExternalInput")
    xab_t = nc.dram_tensor("xab_t", [DIM, COLS], dt.bfloat16, kind="ExternalInput")
    wqkv_t = nc.dram_tensor("wqkv_t", [DIM, 3 * MD], dt.float8e4, kind="ExternalInput")
    wv_t = nc.dram_tensor("wv_t", [DIM, MD], dt.float8e4, kind="ExternalInput")
    wproj_t = nc.dram_tensor("wproj_t", [MD, DIM], dt.float8e4, kind="ExternalInput")
    mask_d = nc.dram_tensor("mask", [128, 128], dt.bfloat16, kind="ExternalInput")
    mscale_d = nc.dram_tensor("mscale", [128, 8], dt.float32, kind="ExternalInput")
    # selC[p,hc,j] = (j==hc): one-hot columns; routes a column-sum matmul's
    # output to row hc of an 8-row PSUM strip (out base partition 32*slab).
    selc_d = nc.dram_tensor("selc", [128, 8, 8], dt.bfloat16, kind="ExternalInput")
    # selR[p,h,j] = (p%32==h): one-hot rows; K=8 matmul broadcasting row h of
    # an [8,512] strip (at base partition 32*slab) across 128 partitions.
    selr_d = nc.dram_tensor("selr", [96, 8, 128], dt.bfloat16, kind="ExternalInput")
    out_t = nc.dram_tensor("out_t", [DIM, COLS], dt.bfloat16, kind="ExternalOutput")

    with tile.TileContext(nc) as tc:
        with (
            tc.tile_pool(name="w", bufs=1) as wpool,
            tc.tile_pool(name="x", bufs=2) as xpool,
            tc.tile_pool(name="slab", bufs=2) as spool,
            tc.tile_pool(name="vslab", bufs=1) as vpool,
            tc.tile_pool(name="nrm", bufs=2) as npool,
            tc.tile_pool(name="att", bufs=3) as fpool,
            tc.tile_pool(name="rtp", bufs=6) as rtpool,
            tc.tile_pool(name="y", bufs=2) as ypool,
            tc.tile_pool(name="mm", bufs=2, space="PSUM") as mmpool,
            tc.tile_pool(name="sc", bufs=3, space="PSUM") as scpool,
            tc.tile_pool(name="sq", bufs=1, space="PSUM") as sqpool,
        ):
            # Table set 6 'natural_log_exp_and_others' holds every ACT
            # function used below (Ln, Exp, Relu, Copy): no table reloads.
            nc.scalar.add_instruction(mybir.InstLoadActFuncSet(
                name=nc.get_next_instruction_name(), act_func_set_id=6,
                ins=[], outs=[]))

            # ---- resident weights/constants ----
            wqkv_sb = wpool.tile([128, 8, 3 * MD], dt.float8e4)
            nc.sync.dma_start(
                wqkv_sb, wqkv_t[:].rearrange("(dc p) f -> p dc f", p=128)
            )
            wv_sb = wpool.tile([128, 8, MD], dt.float8e4)
            nc.sync.dma_start(wv_sb, wv_t[:].rearrange("(dc p) f -> p dc f", p=128))
            wproj_sb = wpool.tile([128, 8, DIM], dt.float8e4)
            nc.sync.dma_start(
                wproj_sb, wproj_t[:].rearrange("(mc p) f -> p mc f", p=128)
            )
            mask_sb = wpool.tile([128, 128], dt.bfloat16)
            nc.sync.dma_start(mask_sb, mask_d[:])
            mscale_sb = wpool.tile([128, 8], dt.float32)
            nc.sync.dma_start(mscale_sb, mscale_d[:])
            selc_sb = wpool.tile([128, 8, 8], dt.bfloat16)
            nc.sync.dma_start(selc_sb, selc_d[:])
            selr_sb = wpool.tile([96, 8, 128], dt.bfloat16)
            nc.sync.dma_start(selr_sb, selr_d[:])

            xe_dram = xe8_t[:].rearrange("(dc p) c -> p dc c", p=128)
            xr_dram = xr8_t[:].rearrange("(dc p) c -> p dc c", p=128)
            xa_dram = xab_t[:].rearrange("(dc p) c -> p dc c", p=128)
            yo_dram = out_t[:].rearrange("(dc p) c -> p dc c", p=128)

            def build_gemm_items(blk):
                """Allocate block tiles + return GEMM work-item closures."""
                c0 = blk * CPB
                xe8 = xpool.tile([128, 8, CPB], dt.float8e4, tag="xe8", name="xe8")
                xr8 = xpool.tile([128, 8, CPB], dt.float8e4, tag="xr8", name="xr8")
                xab = xpool.tile([128, 8, CPB], dt.bfloat16, tag="xab", name="xab")
                nc.sync.dma_start(xe8, xe_dram[:, :, c0 : c0 + CPB])
                nc.sync.dma_start(xr8, xr_dram[:, :, c0 : c0 + CPB])
                nc.sync.dma_start(xab, xa_dram[:, :, c0 : c0 + CPB])
                qT = spool.tile([128, 8, CPB], dt.bfloat16, tag="qT")
                kTa = spool.tile([128, 8, CPB], dt.bfloat16, tag="kTa")
                kTr = spool.tile([128, 8, CPB], dt.bfloat16, tag="kTr")
                va = vpool.tile([128, 4, 8, HD], dt.bfloat16, tag="va")
                vr = vpool.tile([128, 4, 8, HD], dt.bfloat16, tag="vr")
                # per 128-feature-chunk sum-of-squares, one 8-row strip per
                # slab at base partitions 0/32/64 of a single PSUM bank,
                # written by TensorE one-hot-column matmuls.
                ssq = sqpool.tile([96, CPB], dt.float32, tag="ssq")
                st = dict(xe8=xe8, xr8=xr8, xab=xab, qT=qT, kTa=kTa, kTr=kTr,
                          va=va, vr=vr, ssq=ssq, c0=c0)
                slabs = [qT, kTa, kTr]
                items = []

                def qkv_item(fp):
                    # computes fc = 2*fp, 2*fp+1 (same slab)
                    def go():
                        ps = mmpool.tile([128, 2, 512], dt.float32, tag="mmps")
                        for half in range(2):
                            fc = 2 * fp + half
                            src = xr8 if fc >= 16 else xe8
                            for j in range(4):
                                nc.tensor.matmul(
                                    ps[:, half, :],
                                    wqkv_sb[:, 2 * j : 2 * j + 2,
                                            fc * 128 : (fc + 1) * 128],
                                    src[:, 2 * j : 2 * j + 2, :],
                                    start=(j == 0),
                                    stop=(j == 3),
                                    perf_mode=DR,
                                )
                        fc0 = 2 * fp
                        dst = slabs[fc0 // 8][:, fc0 % 8 : fc0 % 8 + 2, :]
                        nc.scalar.copy(out=dst, in_=ps)
                        with nc.allow_low_precision(
                            reason="bf16 squares; PE sums in fp32, ~0.4% rms"
                        ):
                            sq = npool.tile([128, 2, CPB], dt.bfloat16, tag="nsq")
                            nc.vector.tensor_mul(sq, dst, dst)
                            s = fc0 // 8
                            for half in range(2):
                                hc = fc0 % 8 + half
                                nc.tensor.matmul(
                                    ssq[32 * s : 32 * s + 8, :],
                                    selc_sb[:, hc, :],
                                    sq[:, half, :],
                                    start=(hc == 0), stop=(hc == 7),
                                    skip_group_check=True,
                                )
                    return go

                def v_item(isart, rc):
                    def go():
                        src, dstv = (xe8, va) if isart else (xr8, vr)
                        ps = mmpool.tile([128, 2, 512], dt.float32, tag="mmps")
                        for vh in range(2):
                            for j in range(4):
                                nc.tensor.matmul(
                                    ps[:, vh, :],
                                    src[:, 2 * j : 2 * j + 2,
                                        rc * 128 : (rc + 1) * 128],
                                    wv_sb[:, 2 * j : 2 * j + 2,
                                          vh * 512 : (vh + 1) * 512],
                                    start=(j == 0),
                                    stop=(j == 3),
                                    perf_mode=DR,
                                )
                        dv = dstv[:, rc, :, :]
                        nc.scalar.copy(out=dv, in_=ps)
                    return go

                for fp in range(12):
                    items.append(qkv_item(fp))
                for isart in (True, False):
                    for rc in range(4):
                        items.append(v_item(isart, rc))
                return st, items

            def build_attn_items(st):
                """Work items for norm + attention + proj of a block."""
                qT, kTa, kTr = st["qT"], st["kTa"], st["kTr"]
                va, vr, xab, c0 = st["va"], st["vr"], st["xab"], st["c0"]
                ssq = st["ssq"]
                slabs = [qT, kTa, kTr]
                items = []

                # rsqrt of the whole block's ssq in three small ACT pairs
                # (one per 8-row strip): rs = exp(-0.5*ln(scale*ssq));
                # scale=1/HD on the q strip folds the HD**-0.5 score scaling
                # (v12 scheme, fp8 descale cancels).
                lnm = npool.tile([96, CPB], dt.bfloat16, tag="lnm")
                rs = npool.tile([96, CPB], dt.bfloat16, tag="rs")

                def rsqrt_item():
                    with nc.allow_low_precision(
                        reason="bf16 ln/exp of sum-of-squares; ~0.4% rms"
                    ):
                        for s in range(3):
                            sl = slice(32 * s, 32 * s + 8)
                            nc.scalar.activation(
                                lnm[sl, :], ssq[sl, :], Act.Ln,
                                scale=(1.0 / HD if s == 0 else 1.0),
                            )
                            nc.scalar.activation(
                                rs[sl, :], lnm[sl, :], Act.Exp, scale=-0.5
                            )

                def norm_item(s, hp):
                    # scales head-chunks 2*hp, 2*hp+1 of slab s by rs rows
                    # broadcast across partitions via K=8 one-hot-row matmuls.
                    def go():
                        rsb = mmpool.tile([128, 2, 512], dt.float32, tag="mmps")
                        psl = slice(32 * s, 32 * s + 8)
                        for half in range(2):
                            h = 2 * hp + half
                            nc.tensor.matmul(
                                rsb[:, half, :],
                                selr_sb[psl, h, :],
                                rs[psl, :],
                                start=True, stop=True,
                            )
                        with nc.allow_low_precision(
                            reason="bf16 norm scale apply; ~0.4% rms"
                        ):
                            sl = slabs[s][:, 2 * hp : 2 * hp + 2, :]
                            nc.vector.tensor_mul(sl, sl, rsb)
                    return go

                mixedT = vpool.tile([128, 8, CPB], dt.float8e4, tag="mixedT")
                st["mixedT"] = mixedT
                routes = {}

                def scores_item(g, half, hh):
                    def go():
                        gsl = slice(g * 128, (g + 1) * 128)
                        kT = kTa if half == 0 else kTr
                        ps = scpool.tile([128, 4, 128], dt.float32, tag="scps")
                        for i in range(4):
                            h = 4 * hh + i
                            nc.tensor.matmul(
                                ps[:, i, :], kT[:, h, gsl], qT[:, h, gsl],
                                start=True, stop=True,
                            )
                        esc = fpool.tile([128, 4, 128], dt.bfloat16, tag="ers")
                        rsc = fpool.tile([128, 4, 128], dt.bfloat16, tag="ers")
                        nc.scalar.activation(esc, ps, Act.Exp)
                        nc.scalar.activation(rsc, ps, Act.Relu)
                        # elu = relu(s) + (min(exp(s),1) - 1), then *mask/SC
                        route = rtpool.tile([128, 4, 128], dt.bfloat16, tag="rt")
                        with nc.allow_low_precision(
                            reason="bf16 elu combine; ~0.4% rms"
                        ):
                            nc.vector.scalar_tensor_tensor(
                                out=route, in0=esc, scalar=1.0, in1=rsc,
                                op0=Alu.min, op1=Alu.add,
                            )
                            nc.vector.scalar_tensor_tensor(
                                out=route, in0=route, scalar=-1.0,
                                in1=mask_sb[:, None, :].to_broadcast((128, 4, 128)),
                                op0=Alu.add, op1=Alu.mult,
                            )
                        routes[(g, half, hh)] = route
                    return go

                def mixed_item(g, hh):
                    def go():
                        gsl = slice(g * 128, (g + 1) * 128)
                        mx = scpool.tile([128, 4, 128], dt.float32, tag="scps")
                        for i in range(4):
                            h = 4 * hh + i
                            nc.tensor.matmul(
                                mx[:, i, :], va[:, g, h, :],
                                routes[(g, 0, hh)][:, i, :],
                                start=True, stop=False,
                            )
                            nc.tensor.matmul(
                                mx[:, i, :], vr[:, g, h, :],
                                routes[(g, 1, hh)][:, i, :],
                                start=False, stop=True,
                            )
                        nc.scalar.copy(
                            out=mixedT[:, 4 * hh : 4 * hh + 4, gsl], in_=mx
                        )
                    return go

                def proj_item(dp):
                    # projects feature chunks dc = 2*dp, 2*dp+1
                    def go():
                        ps = mmpool.tile([128, 2, 512], dt.float32, tag="mmps")
                        yb = ypool.tile([128, 2, CPB], dt.bfloat16, tag="yb")
                        for half in range(2):
                            dc = 2 * dp + half
                            for j in range(4):
                                nc.tensor.matmul(
                                    ps[:, half, :],
                                    wproj_sb[:, 2 * j : 2 * j + 2,
                                             dc * 128 : (dc + 1) * 128],
                                    mixedT[:, 2 * j : 2 * j + 2, :],
                                    start=(j == 0), stop=(j == 3),
                                    perf_mode=DR,
                                )
                            nc.vector.scalar_tensor_tensor(
                                out=yb[:, half, :], in0=ps[:, half, :],
                                scalar=mscale_sb[:, dc : dc + 1],
                                in1=xab[:, dc, :],
                                op0=Alu.mult, op1=Alu.add,
                            )
                        nc.sync.dma_start(
                            yo_dram[:, 2 * dp : 2 * dp + 2, c0 : c0 + CPB], yb
                        )
                    return go

                items.append(rsqrt_item)
                for s in range(3):
                    for hp in range(4):
                        items.append(norm_item(s, hp))
                for g in range(4):
                    for half in range(2):
                        for hh in range(2):
                            items.append(scores_item(g, half, hh))
                    for hh in range(2):
                        items.append(mixed_item(g, hh))
                for dp in range(4):
                    items.append(proj_item(dp))
                return items

            def merge(attn, gemm):
                """Proportional interleave; rsqrt_item always first so the
                ssq PSUM bank frees before the next block's column-sums."""
                if attn is None:
                    return list(gemm)
                if not gemm:
                    return list(attn)
                out = [attn[0]]
                rest = attn[1:]
                gq = list(gemm)
                na, ng = len(rest), len(gq)
                ai = gi = 0
                while ai < na or gi < ng:
                    # keep attn/gemm issue proportional
                    if gi * na <= ai * ng and gi < ng:
                        out.append(gq[gi]); gi += 1
                    elif ai < na:
                        out.append(rest[ai]); ai += 1
                    else:
                        out.append(gq[gi]); gi += 1
                return out

            blklist = [b for _ in range(repeat) for b in range(nb)]
            prev_st = None
            for i in range(len(blklist) + 1):
                gemm_items = []
                if i < len(blklist):
                    st, gemm_items = build_gemm_items(blklist[i])
                attn = build_attn_items(prev_st) if prev_st is not None else None
                for item in merge(attn, gemm_items):
                    item()
                if i < len(blklist):
                    prev_st = st

    nc.compile()
    return nc


def host_prep(x, artery_embed, residual_kv, Wqkv, Wproj, mixer_scale,
              tok_per_core=TOK_PER_CORE, n_cores=N_CORES):
    T = x.shape[0] * x.shape[1]
    x_flat = np.asarray(x, dtype=np.float32).reshape(T, A, DIM)
    res_flat = np.asarray(residual_kv, dtype=np.float32).reshape(T, RKV, DIM)
    emb = np.asarray(artery_embed, dtype=np.float32)
    xe_flat = x_flat + emb[None]

    Rm = _rope_matrix()
    Wq = np.asarray(Wqkv[0:MD], dtype=np.float64)
    Wk = np.asarray(Wqkv[MD : 2 * MD], dtype=np.float64)
    Wv = np.asarray(Wqkv[2 * MD : 3 * MD], dtype=np.float64)
    Wk_res = np.einsum("de,hec->hdc", Rm, Wk.reshape(HEADS, HD, DIM)).reshape(MD, DIM)

    wqkv_t = np.ascontiguousarray(
        np.concatenate([Wq, Wk, Wk_res], axis=0).T * WSCALE
    ).astype(f8e4)
    wv_t = np.ascontiguousarray(Wv.T * WSCALE).astype(f8e4)
    wproj_t = np.ascontiguousarray(
        np.asarray(Wproj, dtype=np.float64).T * WSCALE
    ).astype(f8e4)

    mask = np.zeros((128, 128), dtype=np.float32)
    for t in range(16):
        mask[t * 8 : (t + 1) * 8, t * 8 : (t + 1) * 8] = 1.0 / SC
    mask = mask.astype(bf16)

    # v path and proj each carry WSCALE; descale both via mscale.
    mscale = np.ascontiguousarray(
        (np.asarray(mixer_scale, dtype=np.float32) / (WSCALE * WSCALE))
        .reshape(8, 128).T
    )

    selc = np.zeros((128, 8, 8), dtype=np.float32)
    selc[:, np.arange(8), np.arange(8)] = 1.0
    selr = np.zeros((96, 8, 128), dtype=np.float32)
    p = np.arange(96)
    for h in range(8):
        selr[p[p % 32 == h], h, :] = 1.0

    shared = dict(
        wqkv_t=wqkv_t, wv_t=wv_t, wproj_t=wproj_t, mask=mask, mscale=mscale,
        selc=selc.astype(bf16), selr=selr.astype(bf16),
    )
    in_maps = []
    for i in range(n_cores):
        sl = slice(i * tok_per_core, (i + 1) * tok_per_core)
        xe = np.ascontiguousarray(
            xe_flat[sl].reshape(tok_per_core * A, DIM).T
        ).astype(f8e4)
        xr = np.ascontiguousarray(
            res_flat[sl].reshape(tok_per_core * RKV, DIM).T
        ).astype(f8e4)
        xa = np.ascontiguousarray(
            x_flat[sl].reshape(tok_per_core * A, DIM).T
        ).astype(bf16)
        m = dict(shared)
        m["xe8_t"] = xe
        m["xr8_t"] = xr
        m["xab_t"] = xa
        in_maps.append(m)
    return in_maps


def assemble_output(outs, tok_per_core=TOK_PER_CORE):
    """outs: list of (DIM, tok_per_core*8) bf16 arrays -> (B,S,A,DIM) f32."""
    parts = []
    for o in outs:
        y = np.asarray(o, dtype=np.float32)  # (1024, T*8)
        parts.append(y.reshape(DIM, tok_per_core, A).transpose(1, 2, 0))
    full = np.concatenate(parts, axis=0)  # (n_tok, A, DIM)
    if full.shape[0] == B * S:
        full = full.reshape(B, S, A, DIM)
    return np.ascontiguousarray(full)


_NC_CACHE = {}


def kernel(x, artery_embed, residual_kv, Wqkv, Wproj, mixer_scale):
    from concourse.bass_utils import run_bass_kernel_spmd

    key = TOK_PER_CORE
    if key not in _NC_CACHE:
        _NC_CACHE[key] = build_program(TOK_PER_CORE)
    nc = _NC_CACHE[key]

    in_maps = host_prep(x, artery_embed, residual_kv, Wqkv, Wproj, mixer_scale)
    res = run_bass_kernel_spmd(nc, in_maps, core_ids=list(range(N_CORES)))
    outs = [r["out_t"] for r in res.results]
    return assemble_output(outs)


# revision 22
# speedup vs baseline: 1.4462x; 1.2385x over previous
"""ArteryMixer Trainium2 kernel v13: v12 fp8 DoubleRow GEMMs + matmul-based
rmsnorm (no GPSIMD all-reduce).

Per-token math (B=2,S=2048,A=8,R=8,DIM=1024,H=8,HD=128,SC=16):
  qkv = concat(x+emb, res) @ Wqkv.T ; q,k rmsnorm ; k_res roped (folded into W);
  scores=elu(q@k.T/sqrt(HD)) ; mixed = scores@v/16 ; out = x + scale*(mixed@Wproj.T)

Deltas vs v12:
  - rmsnorm sum-of-squares over the partition (HD) dim via TensorE ones-column
    matmuls into a [24,512] PSUM tile (one row per 128-feature chunk), instead
    of 24 GPSIMD partition_all_reduce calls per block (~2-3us each - v12's
    bottleneck engine).  rsqrt = exp(-0.5*ln(.)) batched over the whole block
    in two small ACT ops with a per-partition scale vector (1/HD for q rows).
    The per-column scale is broadcast back across partitions with a K=1
    ones-row matmul and applied by one DVE multiply per chunk.
  - scores/mixed PSUM tiles shrunk to [128,4,128] (1 bank) so PSUM fits:
    mm 2x2 banks + scores 3x1 + ssq 1 = 8 banks.
  - qkv evacuation moved to ACT (PSUM-source is cheaper there); the square
    for the norm is a DVE bf16 multiply on the evacuated slab.
"""

import numpy as np
import ml_dtypes

bf16 = ml_dtypes.bfloat16
f8e4 = ml_dtypes.float8_e4m3fn

HEADS = 8
HD = 128
DIM = 1024
MD = 1024
A = 8
RKV = 8
SC = 16
EPS = 1.1920929e-07
ROPE_BASE = 10000.0
N_CORES = 8
B, S = 2, 2048
TOK_PER_CORE = (B * S) // N_CORES  # 512
BLK_TOK = 64                        # tokens per pipeline block
NB = TOK_PER_CORE // BLK_TOK        # 8 blocks
CPB = BLK_TOK * 8                   # 512 cols per block (token-major, slot-minor)
WSCALE = 16.0                       # host-side fp8 weight scale


def _rope_matrix():
    inv_freq = 1.0 / (ROPE_BASE ** (np.arange(0, HD, 2, dtype=np.float64) / HD))
    c, s = np.cos(inv_freq), np.sin(inv_freq)
    Rm = np.zeros((HD, HD), dtype=np.float64)
    i = np.arange(HD // 2)
    # reference _rope: out1 = x1*c + x2*s ; out2 = -x1*s + x2*c
    Rm[i, i] = c
    Rm[i, i + 64] = s
    Rm[i + 64, i] = -s
    Rm[i + 64, i + 64] = c
    return Rm


def build_program(tok_per_core=TOK_PER_CORE, repeat=1, ablate=()):
    """ablate: timing-only experiment flags (break numerics, keep schedule):
    'norm' drops the whole rmsnorm path, 'elu' drops the Exp/Relu ACT pair,
    'qkv1' collapses qkv/v/proj GEMM K-accumulation from 4 matmuls to 1,
    'attnmm' drops the scores/mixed matmuls."""
    import concourse.bass as bass  # noqa
    import concourse.mybir as mybir
    import concourse.tile as tile
    from concourse import bacc

    dt = mybir.dt
    Alu = mybir.AluOpType
    Act = mybir.ActivationFunctionType
    DR = mybir.MatmulPerfMode.DoubleRow

    nb = tok_per_core // BLK_TOK
    COLS = tok_per_core * 8
    ab_norm = "norm" in ablate
    ab_elu = "elu" in ablate
    ab_qkv1 = "qkv1" in ablate
    ab_attnmm = "attnmm" in ablate
    jsteps = [3] if ab_qkv1 else [0, 1, 2, 3]

    nc = bacc.Bacc(None, target_bir_lowering=False)

    xe8_t = nc.dram_tensor("xe8_t", [DIM, COLS], dt.float8e4, kind="ExternalInput")
    xr8_t = nc.dram_tensor("xr8_t", [DIM, COLS], dt.float8e4, kind="ExternalInput")
    xab_t = nc.dram_tensor("xab_t", [DIM, COLS], dt.bfloat16, kind="ExternalInput")
    wqkv_t = nc.dram_tensor("wqkv_t", [DIM, 3 * MD], dt.float8e4, kind="ExternalInput")
    wv_t = nc.dram_tensor("wv_t", [DIM, MD], dt.float8e4, kind="ExternalInput")
    wproj_t = nc.dram_tensor("wproj_t", [MD, DIM], dt.float8e4, kind="ExternalInput")
    mask_d = nc.dram_tensor("mask", [128, 4, 128], dt.bfloat16, kind="ExternalInput")
    mscale_d = nc.dram_tensor("mscale", [128, 8], dt.float32, kind="ExternalInput")
    # selC[p,hc,j] = (j==hc): one-hot columns; routes a column-sum matmul's
    # output to row hc of an 8-row PSUM strip (out base partition 32*slab).
    selc_d = nc.dram_tensor("selc", [128, 8, 8], dt.bfloat16, kind="ExternalInput")
    # selR[p,h,j] = (p%32==h): one-hot rows; K=8 matmul broadcasting row h of
    # an [8,512] strip (at base partition 32*slab) across 128 partitions.
    selr_d = nc.dram_tensor("selr", [96, 8, 128], dt.bfloat16, kind="ExternalInput")
    out_t = nc.dram_tensor("out_t", [DIM, COLS], dt.bfloat16, kind="ExternalOutput")

    with tile.TileContext(nc) as tc:
        with (
            tc.tile_pool(name="w", bufs=1) as wpool,
            tc.tile_pool(name="x", bufs=2) as xpool,
            tc.tile_pool(name="slab", bufs=2) as spool,
            tc.tile_pool(name="vslab", bufs=1) as vpool,
            tc.tile_pool(name="nrm", bufs=2) as npool,
            tc.tile_pool(name="att", bufs=3) as fpool,
            tc.tile_pool(name="rtp", bufs=6) as rtpool,
            tc.tile_pool(name="y", bufs=2) as ypool,
            tc.tile_pool(name="mm", bufs=2, space="PSUM") as mmpool,
            tc.tile_pool(name="sc", bufs=3, space="PSUM") as scpool,
            tc.tile_pool(name="sq", bufs=1, space="PSUM") as sqpool,
        ):
            # Table set 6 'natural_log_exp_and_others' holds every ACT
            # function used below (Ln, Exp, Relu, Copy): no table reloads.
            nc.scalar.add_instruction(mybir.InstLoadActFuncSet(
                name=nc.get_next_instruction_name(), act_func_set_id=6,
                ins=[], outs=[]))

            # ---- resident weights/constants ----
            wqkv_sb = wpool.tile([128, 8, 3 * MD], dt.float8e4)
            nc.sync.dma_start(
                wqkv_sb, wqkv_t[:].rearrange("(dc p) f -> p dc f", p=128)
            )
            wv_sb = wpool.tile([128, 8, MD], dt.float8e4)
            nc.sync.dma_start(wv_sb, wv_t[:].rearrange("(dc p) f -> p dc f", p=128))
            wproj_sb = wpool.tile([128, 8, DIM], dt.float8e4)
            nc.sync.dma_start(
                wproj_sb, wproj_t[:].rearrange("(mc p) f -> p mc f", p=128)
            )
            # [128,4,128] so the elu-combine reads it with a plain packed AP
            # (a broadcast AP would drop DVE to 1x mode)
            mask_sb = wpool.tile([128, 4, 128], dt.bfloat16)
            nc.sync.dma_start(mask_sb, mask_d[:])
            mscale_sb = wpool.tile([128, 8], dt.float32)
            nc.sync.dma_start(mscale_sb, mscale_d[:])
            selc_sb = wpool.tile([128, 8, 8], dt.bfloat16)
            nc.sync.dma_start(selc_sb, selc_d[:])
            selr_sb = wpool.tile([96, 8, 128], dt.bfloat16)
            nc.sync.dma_start(selr_sb, selr_d[:])

            xe_dram = xe8_t[:].rearrange("(dc p) c -> p dc c", p=128)
            xr_dram = xr8_t[:].rearrange("(dc p) c -> p dc c", p=128)
            xa_dram = xab_t[:].rearrange("(dc p) c -> p dc c", p=128)
            yo_dram = out_t[:].rearrange("(dc p) c -> p dc c", p=128)

            def build_gemm_steps(blk):
                """GEMM + rmsnorm of one block as 20 software-pipelined steps.

                Step c runs: its own GEMM matmuls + PSUM evacuation, then the
                cross-engine consumers of EARLIER steps (square at lag 1,
                column-sum matmul at lag 2, ln/exp one step after a strip
                completes, broadcast+apply one step after that).  The lags
                keep every instruction ready by the time its in-order engine
                queue reaches it — emitting a consumer right after its
                producer stalls the consumer's whole engine on the
                producer's latency (v13's critical-path bug).
                """
                c0 = blk * CPB
                xe8 = xpool.tile([128, 8, CPB], dt.float8e4, tag="xe8", name="xe8")
                xr8 = xpool.tile([128, 8, CPB], dt.float8e4, tag="xr8", name="xr8")
                xab = xpool.tile([128, 8, CPB], dt.bfloat16, tag="xab", name="xab")
                nc.sync.dma_start(xe8, xe_dram[:, :, c0 : c0 + CPB])
                nc.sync.dma_start(xr8, xr_dram[:, :, c0 : c0 + CPB])
                nc.sync.dma_start(xab, xa_dram[:, :, c0 : c0 + CPB])
                qT = spool.tile([128, 8, CPB], dt.bfloat16, tag="qT")
                kTa = spool.tile([128, 8, CPB], dt.bfloat16, tag="kTa")
                kTr = spool.tile([128, 8, CPB], dt.bfloat16, tag="kTr")
                va = vpool.tile([128, 4, 8, HD], dt.bfloat16, tag="va")
                vr = vpool.tile([128, 4, 8, HD], dt.bfloat16, tag="vr")
                # per 128-feature-chunk sum-of-squares, one 8-row strip per
                # slab at base partitions 0/32/64 of a single PSUM bank,
                # written by TensorE one-hot-column matmuls.
                ssq = sqpool.tile([96, CPB], dt.float32, tag="ssq")
                lnm = npool.tile([96, CPB], dt.bfloat16, tag="lnm")
                rs = npool.tile([96, CPB], dt.bfloat16, tag="rs")
                st = dict(xe8=xe8, xr8=xr8, xab=xab, qT=qT, kTa=kTa, kTr=kTr,
                          va=va, vr=vr, c0=c0)
                slabs = [qT, kTa, kTr]
                sqtiles = {}

                def qkv_main(fp):
                    # computes fc = 2*fp, 2*fp+1 (same slab)
                    ps = mmpool.tile([128, 2, 512], dt.float32, tag="mmps")
                    for half in range(2):
                        fc = 2 * fp + half
                        src = xr8 if fc >= 16 else xe8
                        for j in jsteps:
                            nc.tensor.matmul(
                                ps[:, half, :],
                                wqkv_sb[:, 2 * j : 2 * j + 2,
                                        fc * 128 : (fc + 1) * 128],
                                src[:, 2 * j : 2 * j + 2, :],
                                start=(j == jsteps[0]),
                                stop=(j == 3),
                                perf_mode=DR,
                            )
                    fc0 = 2 * fp
                    dst = slabs[fc0 // 8][:, fc0 % 8 : fc0 % 8 + 2, :]
                    nc.scalar.copy(out=dst, in_=ps)

                def v_main(vi):
                    isart, rc = vi // 4, vi % 4
                    src, dstv = (xe8, va) if isart == 0 else (xr8, vr)
                    ps = mmpool.tile([128, 2, 512], dt.float32, tag="mmps")
                    for vh in range(2):
                        for j in jsteps:
                            nc.tensor.matmul(
                                ps[:, vh, :],
                                src[:, 2 * j : 2 * j + 2,
                                    rc * 128 : (rc + 1) * 128],
                                wv_sb[:, 2 * j : 2 * j + 2,
                                      vh * 512 : (vh + 1) * 512],
                                start=(j == jsteps[0]),
                                stop=(j == 3),
                                perf_mode=DR,
                            )
                    nc.scalar.copy(out=dstv[:, rc, :, :], in_=ps)

                def sq_fn(fp):
                    fc0 = 2 * fp
                    dst = slabs[fc0 // 8][:, fc0 % 8 : fc0 % 8 + 2, :]
                    with nc.allow_low_precision(
                        reason="bf16 squares; PE sums in fp32, ~0.4% rms"
                    ):
                        sq = npool.tile([128, 2, CPB], dt.bfloat16, tag="nsq")
                        nc.vector.tensor_mul(sq, dst, dst)
                    sqtiles[fp] = sq

                def cs_fn(fp):
                    fc0 = 2 * fp
                    s = fc0 // 8
                    sq = sqtiles.pop(fp)
                    for half in range(2):
                        hc = fc0 % 8 + half
                        nc.tensor.matmul(
                            ssq[32 * s : 32 * s + 8, :],
                            selc_sb[:, hc, :],
                            sq[:, half, :],
                            start=(hc == 0), stop=(hc == 7),
                            skip_group_check=True,
                        )

                def lnexp_fn(s):
                    # rs = exp(-0.5*ln(scale*ssq)); scale=1/HD on the q strip
                    # folds the HD**-0.5 score scale (fp8 descale cancels).
                    with nc.allow_low_precision(
                        reason="bf16 ln/exp of sum-of-squares; ~0.4% rms"
                    ):
                        sl = slice(32 * s, 32 * s + 8)
                        nc.scalar.activation(
                            lnm[sl, :], ssq[sl, :], Act.Ln,
                            scale=(1.0 / HD if s == 0 else 1.0),
                        )
                        nc.scalar.activation(
                            rs[sl, :], lnm[sl, :], Act.Exp, scale=-0.5
                        )

                def norm_fn(s, hp):
                    # scales head-chunks 2*hp, 2*hp+1 of slab s by rs rows
                    # broadcast across partitions via K=8 one-hot-row matmuls.
                    rsb = mmpool.tile([128, 2, 512], dt.float32, tag="mmps")
                    psl = slice(32 * s, 32 * s + 8)
                    for half in range(2):
                        h = 2 * hp + half
                        nc.tensor.matmul(
                            rsb[:, half, :],
                            selr_sb[psl, h, :],
                            rs[psl, :],
                            start=True, stop=True,
                        )
                    with nc.allow_low_precision(
                        reason="bf16 norm scale apply; ~0.4% rms"
                    ):
                        sl = slabs[s][:, 2 * hp : 2 * hp + 2, :]
                        nc.vector.tensor_mul(sl, sl, rsb)

                def step(c):
                    def go():
                        if c < 12:
                            qkv_main(c)
                        else:
                            v_main(c - 12)
                        if not ab_norm:
                            if 1 <= c <= 12:
                                sq_fn(c - 1)
                            if 2 <= c <= 13:
                                cs_fn(c - 2)
                            if c in (5, 9, 13):
                                lnexp_fn((c - 5) // 4)
                            if 6 <= c <= 17:
                                s, hp = divmod(c - 6, 4)
                                norm_fn(s, hp)
                    return go

                return st, [step(c) for c in range(20)]

            def build_attn_steps(st):
                """Scores/mixed/proj of a block (slabs already normalized in
                its gemm phase).  Each unit flushes the previous unit's
                cross-engine tail first (lag 1), then runs its matmuls."""
                qT, kTa, kTr = st["qT"], st["kTa"], st["kTr"]
                va, vr, xab, c0 = st["va"], st["vr"], st["xab"], st["c0"]
                mixedT = vpool.tile([128, 8, CPB], dt.float8e4, tag="mixedT")
                routes = {}
                pending = [None]

                def unit(main, post):
                    def go():
                        p, pending[0] = pending[0], None
                        if p:
                            p()
                        main()
                        pending[0] = post
                    return go

                def scores_unit(g, half, hh):
                    gsl = slice(g * 128, (g + 1) * 128)
                    kT = kTa if half == 0 else kTr
                    cell = {}

                    def main():
                        ps = scpool.tile([128, 4, 128], dt.float32, tag="scps")
                        for i in range(4) if not ab_attnmm else ():
                            h = 4 * hh + i
                            nc.tensor.matmul(
                                ps[:, i, :], kT[:, h, gsl], qT[:, h, gsl],
                                start=True, stop=True,
                            )
                        esc = fpool.tile([128, 4, 128], dt.bfloat16, tag="ers")
                        rsc = fpool.tile([128, 4, 128], dt.bfloat16, tag="ers")
                        nc.scalar.activation(esc, ps, Act.Exp)
                        if ab_elu:
                            rsc = esc
                        else:
                            nc.scalar.activation(rsc, ps, Act.Relu)
                        cell["esc"], cell["rsc"] = esc, rsc

                    def post():
                        # elu = relu(s) + (min(exp(s),1) - 1), then *mask/SC
                        route = rtpool.tile([128, 4, 128], dt.bfloat16, tag="rt")
                        with nc.allow_low_precision(
                            reason="bf16 elu combine; ~0.4% rms"
                        ):
                            nc.vector.scalar_tensor_tensor(
                                out=route, in0=cell["esc"], scalar=1.0,
                                in1=cell["rsc"], op0=Alu.min, op1=Alu.add,
                            )
                            nc.vector.scalar_tensor_tensor(
                                out=route, in0=route, scalar=-1.0,
                                in1=mask_sb, op0=Alu.add, op1=Alu.mult,
                            )
                        routes[(g, half, hh)] = route

                    return unit(main, post)

                def mixed_unit(g, hh):
                    gsl = slice(g * 128, (g + 1) * 128)
                    cell = {}

                    def main():
                        mx = scpool.tile([128, 4, 128], dt.float32, tag="scps")
                        for i in range(4) if not ab_attnmm else ():
                            h = 4 * hh + i
                            nc.tensor.matmul(
                                mx[:, i, :], va[:, g, h, :],
                                routes[(g, 0, hh)][:, i, :],
                                start=True, stop=False,
                            )
                            nc.tensor.matmul(
                                mx[:, i, :], vr[:, g, h, :],
                                routes[(g, 1, hh)][:, i, :],
                                start=False, stop=True,
                            )
                        cell["mx"] = mx

                    def post():
                        nc.scalar.copy(
                            out=mixedT[:, 4 * hh : 4 * hh + 4, gsl],
                            in_=cell["mx"],
                        )

                    return unit(main, post)

                def proj_unit(dp):
                    # projects feature chunks dc = 2*dp, 2*dp+1
                    cell = {}

                    def main():
                        ps = mmpool.tile([128, 2, 512], dt.float32, tag="mmps")
                        for half in range(2):
                            dc = 2 * dp + half
                            for j in jsteps:
                                nc.tensor.matmul(
                                    ps[:, half, :],
                                    wproj_sb[:, 2 * j : 2 * j + 2,
                                             dc * 128 : (dc + 1) * 128],
                                    mixedT[:, 2 * j : 2 * j + 2, :],
                                    start=(j == jsteps[0]), stop=(j == 3),
                                    perf_mode=DR,
                                )
                        cell["ps"] = ps

                    def post():
                        ps = cell["ps"]
                        yb = ypool.tile([128, 2, CPB], dt.bfloat16, tag="yb")
                        for half in range(2):
                            dc = 2 * dp + half
                            nc.vector.scalar_tensor_tensor(
                                out=yb[:, half, :], in0=ps[:, half, :],
                                scalar=mscale_sb[:, dc : dc + 1],
                                in1=xab[:, dc, :],
                                op0=Alu.mult, op1=Alu.add,
                            )
                        nc.sync.dma_start(
                            yo_dram[:, 2 * dp : 2 * dp + 2, c0 : c0 + CPB], yb
                        )

                    return unit(main, post)

                def flush():
                    p, pending[0] = pending[0], None
                    if p:
                        p()

                steps = []
                for g in range(4):
                    for half in range(2):
                        for hh in range(2):
                            steps.append(scores_unit(g, half, hh))
                    for hh in range(2):
                        steps.append(mixed_unit(g, hh))
                for dp in range(4):
                    steps.append(proj_unit(dp))
                steps.append(flush)
                return steps

            def merge(attn, gemm):
                """Proportional interleave of the two step lists."""
                if not attn:
                    return list(gemm)
                if not gemm:
                    return list(attn)
                out = []
                na, ng = len(attn), len(gemm)
                ai = gi = 0
                while ai < na or gi < ng:
                    if gi * na <= ai * ng and gi < ng:
                        out.append(gemm[gi]); gi += 1
                    elif ai < na:
                        out.append(attn[ai]); ai += 1
                    else:
                        out.append(gemm[gi]); gi += 1
                return out

            blklist = [b for _ in range(repeat) for b in range(nb)]
            prev_st = None
            for i in range(len(blklist) + 1):
                gemm_steps = []
                if i < len(blklist):
                    st, gemm_steps = build_gemm_steps(blklist[i])
                attn = build_attn_steps(prev_st) if prev_st is not None else []
                for item in merge(attn, gemm_steps):
                    item()
                if i < len(blklist):
                    prev_st = st

    nc.compile()
    return nc


def host_prep(x, artery_embed, residual_kv, Wqkv, Wproj, mixer_scale,
              tok_per_core=TOK_PER_CORE, n_cores=N_CORES):
    T = x.shape[0] * x.shape[1]
    x_flat = np.asarray(x, dtype=np.float32).reshape(T, A, DIM)
    res_flat = np.asarray(residual_kv, dtype=np.float32).reshape(T, RKV, DIM)
    emb = np.asarray(artery_embed, dtype=np.float32)
    xe_flat = x_flat + emb[None]

    Rm = _rope_matrix()
    Wq = np.asarray(Wqkv[0:MD], dtype=np.float64)
    Wk = np.asarray(Wqkv[MD : 2 * MD], dtype=np.float64)
    Wv = np.asarray(Wqkv[2 * MD : 3 * MD], dtype=np.float64)
    Wk_res = np.einsum("de,hec->hdc", Rm, Wk.reshape(HEADS, HD, DIM)).reshape(MD, DIM)

    wqkv_t = np.ascontiguousarray(
        np.concatenate([Wq, Wk, Wk_res], axis=0).T * WSCALE
    ).astype(f8e4)
    wv_t = np.ascontiguousarray(Wv.T * WSCALE).astype(f8e4)
    wproj_t = np.ascontiguousarray(
        np.asarray(Wproj, dtype=np.float64).T * WSCALE
    ).astype(f8e4)

    mask = np.zeros((128, 128), dtype=np.float32)
    for t in range(16):
        mask[t * 8 : (t + 1) * 8, t * 8 : (t + 1) * 8] = 1.0 / SC
    mask = np.ascontiguousarray(
        np.broadcast_to(mask[:, None, :], (128, 4, 128))
    ).astype(bf16)

    # v path and proj each carry WSCALE; descale both via mscale.
    mscale = np.ascontiguousarray(
        (np.asarray(mixer_scale, dtype=np.float32) / (WSCALE * WSCALE))
        .reshape(8, 128).T
    )

    selc = np.zeros((128, 8, 8), dtype=np.float32)
    selc[:, np.arange(8), np.arange(8)] = 1.0
    selr = np.zeros((96, 8, 128), dtype=np.float32)
    p = np.arange(96)
    for h in range(8):
        selr[p[p % 32 == h], h, :] = 1.0

    shared = dict(
        wqkv_t=wqkv_t, wv_t=wv_t, wproj_t=wproj_t, mask=mask, mscale=mscale,
        selc=selc.astype(bf16), selr=selr.astype(bf16),
    )
    in_maps = []
    for i in range(n_cores):
        sl = slice(i * tok_per_core, (i + 1) * tok_per_core)
        xe = np.ascontiguousarray(
            xe_flat[sl].reshape(tok_per_core * A, DIM).T
        ).astype(f8e4)
        xr = np.ascontiguousarray(
            res_flat[sl].reshape(tok_per_core * RKV, DIM).T
        ).astype(f8e4)
        xa = np.ascontiguousarray(
            x_flat[sl].reshape(tok_per_core * A, DIM).T
        ).astype(bf16)
        m = dict(shared)
        m["xe8_t"] = xe
        m["xr8_t"] = xr
        m["xab_t"] = xa
        in_maps.append(m)
    return in_maps


def assemble_output(outs, tok_per_core=TOK_PER_CORE):
    """outs: list of (DIM, tok_per_core*8) bf16 arrays -> (B,S,A,DIM) f32."""
    parts = []
    for o in outs:
        y = np.asarray(o, dtype=np.float32)  # (1024, T*8)
        parts.append(y.reshape(DIM, tok_per_core, A).transpose(1, 2, 0))
    full = np.concatenate(parts, axis=0)  # (n_tok, A, DIM)
    if full.shape[0] == B * S:
        full = full.reshape(B, S, A, DIM)
    return np.ascontiguousarray(full)


_NC_CACHE = {}


def kernel(x, artery_embed, residual_kv, Wqkv, Wproj, mixer_scale):
    from concourse.bass_utils import run_bass_kernel_spmd

    key = TOK_PER_CORE
    if key not in _NC_CACHE:
        _NC_CACHE[key] = build_program(TOK_PER_CORE)
    nc = _NC_CACHE[key]

    in_maps = host_prep(x, artery_embed, residual_kv, Wqkv, Wproj, mixer_scale)
    res = run_bass_kernel_spmd(nc, in_maps, core_ids=list(range(N_CORES)))
    outs = [r["out_t"] for r in res.results]
    return assemble_output(outs)
